# revision 2
# baseline (speedup 1.0000x reference)
"""DynamicConv1D Trainium2 kernel.

Reference computation (per batch b):
  dw = conv1d(x, W, pad=3) + b            # [O*I*K, T] dynamic weights
  dw = softmax(dw.reshape(O,I,K,T)/sqrt(K), axis=K)
  y[o,t] = sum_{i,k} x[i, t+k-3] * dw[o,i,k,t]

Sharding: 8 cores = 4 batches x 2 halves of O (16 out-channels each).
Each core gets x[b] plus its half of the (rearranged) conv weights and
computes y[b, half*16:(half+1)*16, :]. No collectives; the host scatters
inputs and concatenates outputs.

Per-core layout (t-tile = 128 positions on partitions):
  conv as matmul: dw[t, (k,o,i)] = sum_{(j,c)} X1[(j,c), t] * W'[(j,c), (k,o,i)]
    X1[(j,c), u] = x[c, u+j-3]  (im2col built host-side, bf16); ones row in
    X1b so the bias rides as an extra W' row; 1/sqrt(K) folded into W'/b.
  dw chunks (512 cols = 1 psum bank) are exp'd by ScalarE into an SBUF
  tile eex laid out [t, k, {e,ex}, (o,i)] so the k-sum trees for den/num
  run as wide flat scalar_tensor_tensor ops (DVE 4x perf mode: all
  operands bf16, packed, SBUF).
  EX = e * x_unf via 7 per-k STTs (x broadcast over o with a stride-0
  view; walrus limits STT operands to <=2 free dims).
  Tail per tile-pair: r = 1/den (bf16 reciprocal), y1 = num*r (4x STT),
  y[t,o] = reduce_i y1 (f32 out).
"""

import numpy as np

B = 4
C = 32
K = 7
T = 4096
O_FULL = 32
OH = 16  # out-channels per core
PAD = 3
TT = 128  # t positions per tile (partition dim)
FREE = K * OH * C  # 3584, matmul free index = k*512 + o*32 + i
SLAB = OH * C  # 512, one k-slab
CD1 = 128  # (j, c) rows for j=0..3
CD2 = 97  # (j, c) rows for j=4..6 plus ones row
CHUNK = 512  # psum chunk (1 bank); FREE = 7*CHUNK
KI = K * C  # 224

_prog_cache = {}


def _build(t_len):
    """Build and compile the per-core Bass program for sequence length t_len."""
    import concourse.tile as tile
    from concourse import bacc, mybir

    nt = t_len // TT
    nc = bacc.Bacc("TRN2", target_bir_lowering=False, debug=False, num_devices=1)
    f32 = mybir.dt.float32
    bf16 = mybir.dt.bfloat16
    mult = mybir.AluOpType.mult
    add = mybir.AluOpType.add

    x1a_d = nc.dram_tensor("x1a", [CD1, t_len], bf16, kind="ExternalInput").ap()
    x1b_d = nc.dram_tensor("x1b", [CD2, t_len], bf16, kind="ExternalInput").ap()
    w1_d = nc.dram_tensor("wp1", [CD1, FREE], bf16, kind="ExternalInput").ap()
    w2_d = nc.dram_tensor("wp2", [CD2, FREE], bf16, kind="ExternalInput").ap()
    x2_d = nc.dram_tensor("x2h", [TT, nt * KI], bf16, kind="ExternalInput").ap()
    y_d = nc.dram_tensor("yout", [TT, nt * OH], f32, kind="ExternalOutput").ap()

    def stt(out, in0, in1, op1):
        nc.vector.scalar_tensor_tensor(out, in0, 1.0, in1, op0=mult, op1=op1)

    with tile.TileContext(nc) as tc:
        with (
            tc.tile_pool(name="const", bufs=1) as cpool,
            tc.tile_pool(name="ep", bufs=3) as epool,
            tc.tile_pool(name="tree", bufs=3) as tpool,
            tc.tile_pool(name="small", bufs=2) as spool,
            tc.tile_pool(name="psum", bufs=1, space="PSUM") as ppool,
        ):
            x1a = cpool.tile([CD1, t_len], bf16, tag="x1a")
            x1b = cpool.tile([CD2, t_len], bf16, tag="x1b")
            w1 = cpool.tile([CD1, FREE], bf16, tag="w1")
            w2 = cpool.tile([CD2, FREE], bf16, tag="w2")
            x2h = cpool.tile([TT, nt, KI], bf16, tag="x2h")
            y_sb = cpool.tile([TT, nt * OH], f32, tag="ysb")

            # Initial loads, split across the sync and gpsimd DMA queues with
            # the first tiles' dependencies (w chunks, x1 head, x2h head) first.
            h = t_len // 2
            hf = FREE // 2
            nc.sync.dma_start(x1a[:, 0:h], x1a_d[:, 0:h])
            nc.gpsimd.dma_start(x1b[:, 0:h], x1b_d[:, 0:h])
            nc.sync.dma_start(w1[:, 0:hf], w1_d[:, 0:hf])
            nc.gpsimd.dma_start(w2[:, 0:hf], w2_d[:, 0:hf])
            nc.sync.dma_start(w1[:, hf:], w1_d[:, hf:])
            nc.gpsimd.dma_start(w2[:, hf:], w2_d[:, hf:])
            nhalf = (nt // 2) * KI
            nc.sync.dma_start(
                x2h[:].rearrange("p a b -> p (a b)")[:, 0:nhalf], x2_d[:, 0:nhalf]
            )
            nc.gpsimd.dma_start(
                x2h[:].rearrange("p a b -> p (a b)")[:, nhalf:], x2_d[:, nhalf:]
            )
            nc.sync.dma_start(x1a[:, h:], x1a_d[:, h:])
            nc.gpsimd.dma_start(x1b[:, h:], x1b_d[:, h:])

            for tt in range(nt):
                t0 = tt * TT
                x1at = x1a[:, t0 : t0 + TT]
                x1bt = x1b[:, t0 : t0 + TT]

                # psum: 3 double-bank groups + 1 single bank (7 chunks of 512)
                pg = [
                    ppool.tile([TT, 1024], f32, tag="pA", name="pA"),
                    ppool.tile([TT, 1024], f32, tag="pB", name="pB"),
                    ppool.tile([TT, 1024], f32, tag="pC", name="pC"),
                    ppool.tile([TT, 512], f32, tag="pD", name="pD"),
                ]

                def chunk_ap(ci):
                    g, o = divmod(ci, 2)
                    return pg[g][:, o * 512 : (o + 1) * 512]

                # a-pass (stationary x1a) then b-pass (stationary x1b): one
                # weight load per pass instead of per chunk-matmul pair.
                for ci in range(K):
                    cs = slice(ci * CHUNK, (ci + 1) * CHUNK)
                    nc.tensor.matmul(
                        chunk_ap(ci), x1at, w1[:, cs], start=True, stop=False
                    )
                for ci in range(K):
                    cs = slice(ci * CHUNK, (ci + 1) * CHUNK)
                    nc.tensor.matmul(
                        chunk_ap(ci), x1bt, w2[:, cs], start=False, stop=True
                    )

                # eex[t, k, {e, ex}, (o,i)]
                eex = epool.tile([TT, K, 2, SLAB], bf16, tag="eex")
                for g in range(4):
                    kw = 2 if g < 3 else 1  # k-slabs in this group
                    src = pg[g][:].rearrange("p (k q) -> p k q", k=kw)
                    nc.scalar.activation(
                        eex[:, 2 * g : 2 * g + kw, 0, :],
                        src,
                        mybir.ActivationFunctionType.Exp,
                    )

                # EX_k = e_k * x_unf_k (x broadcast over o)
                for k in range(K):
                    xk = (
                        x2h[:, tt, k * C : (k + 1) * C]
                        .unsqueeze(1)
                        .broadcast_to([TT, OH, C])
                    )
                    stt(
                        eex[:, k, 1, :].rearrange("p (o i) -> p o i", o=OH),
                        eex[:, k, 0, :].rearrange("p (o i) -> p o i", o=OH),
                        xk,
                        mult,
                    )

                # k-sum trees for den (over e) and num (over EX), both halves
                # ride in each wide op via the (sn, q) flattening.
                ev = eex[:].rearrange("p k s q -> p k (s q)")
                t1 = tpool.tile([TT, 3, 2 * SLAB], bf16, tag="t1")
                stt(t1[:], ev[:, 0:6:2], ev[:, 1:6:2], add)
                t2 = tpool.tile([TT, 2 * SLAB], bf16, tag="t2")
                stt(t2[:], t1[:, 0], t1[:, 1], add)
                t3 = tpool.tile([TT, 2 * SLAB], bf16, tag="t3")
                stt(t3[:], t1[:, 2], ev[:, 6], add)
                if tt % 2 == 0:
                    nd2 = spool.tile([TT, 2, 2, SLAB], bf16, tag="nd2")
                stt(nd2[:, tt % 2], t2[:].rearrange("p (s q) -> p s q", s=2),
                    t3[:].rearrange("p (s q) -> p s q", s=2), add)

                if tt % 2 == 1:
                    # softmax tail for the tile pair: r = 1/den (bf16), then
                    # y[t,o] = sum_i num * r
                    r2 = spool.tile([TT, 2, SLAB], bf16, tag="r2")
                    with nc.allow_low_precision("softmax denominator in bf16"):
                        nc.vector.reciprocal(r2[:], nd2[:, :, 0])
                    y1 = spool.tile([TT, 2, SLAB], bf16, tag="y1")
                    stt(y1[:], nd2[:, :, 1], r2[:], mult)
                    nc.vector.tensor_reduce(
                        y_sb[:, (tt - 1) * OH : (tt + 1) * OH],
                        y1[:].rearrange("p u (o i) -> p u o i", o=OH),
                        axis=mybir.AxisListType.X,
                        op=mybir.AluOpType.add,
                    )

                if (tt + 1) % 8 == 0 or tt == nt - 1:
                    g0 = (tt // 8) * 8 * OH
                    nc.gpsimd.dma_start(
                        y_d[:, g0 : (tt + 1) * OH], y_sb[:, g0 : (tt + 1) * OH]
                    )

    nc.compile()
    return nc


def _prep_inputs(x, W, b):
    """Host-side scatter: per-core input dicts (pure layout/slicing)."""
    import ml_dtypes

    bf = ml_dtypes.bfloat16
    scale = np.float32(1.0 / np.sqrt(K))
    halves = []
    for h in range(2):
        Wh = W[h * OH * C * K : (h + 1) * OH * C * K]  # [OH*C*K, C, K]
        # rows (j,c) -> j*32+c ; cols (k,o,i) -> k*512 + o*32 + i
        Wp = (
            Wh.reshape(OH, C, K, C, K).transpose(4, 3, 2, 0, 1).reshape(K * C, FREE)
            * scale
        )
        bh = (
            b[h * OH * C * K : (h + 1) * OH * C * K]
            .reshape(OH, C, K)
            .transpose(2, 0, 1)
            .reshape(FREE)
            * scale
        )
        w1 = np.ascontiguousarray(Wp[:CD1])
        w2 = np.ascontiguousarray(
            np.concatenate([Wp[CD1:], bh[None, :]], axis=0)
        )
        halves.append((w1.astype(bf), w2.astype(bf)))

    t_len = x.shape[-1]
    nt = t_len // TT
    x1s = []
    for bi in range(B):
        xp = np.zeros((C, t_len + 2 * PAD), dtype=np.float32)
        xp[:, PAD : PAD + t_len] = x[bi]
        x1a = np.empty((CD1, t_len), dtype=np.float32)
        x1b = np.empty((CD2, t_len), dtype=np.float32)
        for j in range(K):
            tgt, r0 = (x1a, j * C) if j < 4 else (x1b, (j - 4) * C)
            tgt[r0 : r0 + C] = xp[:, j : j + t_len]
        x1b[CD2 - 1] = 1.0
        # x_unf in [t, (k,i)] order, tiled as [tp, tt, k*32+i]
        xu = np.empty((K, C, t_len), dtype=np.float32)
        for k in range(K):
            xu[k] = xp[:, k : k + t_len]
        x2h = (
            xu.transpose(2, 0, 1)  # [t, k, i]
            .reshape(nt, TT, KI)
            .transpose(1, 0, 2)  # [tp, tt, (k,i)]
            .reshape(TT, nt * KI)
        )
        x1s.append((x1a.astype(bf), x1b.astype(bf), np.ascontiguousarray(x2h).astype(bf)))

    in_maps = []
    for core in range(8):
        bi, h = divmod(core, 2)
        w1, w2 = halves[h]
        x1a, x1b, x2h = x1s[bi]
        in_maps.append({"x1a": x1a, "x1b": x1b, "wp1": w1, "wp2": w2, "x2h": x2h})
    return in_maps


def _assemble(results, t_len):
    """Gather per-core [TT, nt*OH] outputs into [B, O_FULL, t_len]."""
    nt = t_len // TT
    y = np.empty((B, O_FULL, t_len), dtype=np.float32)
    for core, res in enumerate(results):
        bi, h = divmod(core, 2)
        arr = res["yout"].reshape(TT, nt, OH)  # [tp, tt, o]
        y[bi, h * OH : (h + 1) * OH, :] = arr.transpose(2, 1, 0).reshape(OH, t_len)
    return y


def _run(x, W, b, trace=False, trace_cores=None):
    from concourse.bass_utils import run_bass_kernel_spmd
    from concourse.bass_interp import get_hw_module

    t_len = x.shape[-1]
    key = ("prog", t_len)
    if key not in _prog_cache:
        nc = _build(t_len)
        nc.m = get_hw_module(nc.m)
        _prog_cache[key] = nc
    nc = _prog_cache[key]

    in_maps = _prep_inputs(x, W, b)
    res = run_bass_kernel_spmd(
        nc,
        in_maps,
        core_ids=list(range(8)),
        trace=trace,
        trace_cores=trace_cores,
    )
    return _assemble(res.results, t_len), res


def kernel(x, W, b):
    y, _ = _run(np.asarray(x), np.asarray(W), np.asarray(b))
    return y


# revision 3
# speedup vs baseline: 1.8785x; 1.8785x over previous
"""DynamicConv1D Trainium2 kernel.

Reference computation (per batch b):
  dw = conv1d(x, W, pad=3) + b            # [O*I*K, T] dynamic weights
  dw = softmax(dw.reshape(O,I,K,T)/sqrt(K), axis=K)
  y[o,t] = sum_{i,k} x[i, t+k-3] * dw[o,i,k,t]

Sharding: 8 cores = 4 batches x 2 halves of O (16 out-channels each).
Each core gets x[b] plus its half of the (rearranged) conv weights and
computes y[b, half*16:(half+1)*16, :]. No collectives; the host scatters
inputs and concatenates outputs.

Per-core layout (t-tile = 128 positions on partitions):
  conv as matmul: dw[t, (k,o,i)] = sum_{(j,c)} X1[(j,c), t] * W'[(j,c), (k,o,i)]
    X1[(j,c), u] = x[c, u+j-3]  (im2col built host-side, bf16); ones row in
    X1b so the bias rides as an extra W' row; 1/sqrt(K) folded into W'/b.
  psum is organized as 4 bank groups (2+2+2+1 banks) so ScalarE drains dw
  with four wide exp's per tile (amortizes the activation access latency).
  x_unf ships from the host as a [tp, tile, (k,i)] tensor (no on-device DMA
  transposes).  eex layout [t, k, {e,ex}, (o,i)] keeps every k-sum tree
  level a flat tensor_tensor add (DVE 2x mode: all operands bf16, packed).
  Tail per tile-pair: den f32, r = recip_approx_fast(den), y1 = num*r,
  y[t,o] = reduce_i y1 (one 2x halving of i first).
"""

import numpy as np

B = 4
C = 32
K = 7
T = 4096
O_FULL = 32
OH = 16  # out-channels per core
PAD = 3
TT = 128  # t positions per tile (partition dim)
FREE = K * OH * C  # 3584, matmul free index = k*512 + o*32 + i
SLAB = OH * C  # 512, one k-slab
CD1 = 128  # (j, c) rows for j=0..3
CD2 = 97  # (j, c) rows for j=4..6 plus ones row
CHUNK = 512  # psum chunk (1 bank); FREE = 7*CHUNK
KI = K * C  # 224

_prog_cache = {}


def _build(t_len):
    """Build and compile the per-core Bass program for sequence length t_len."""
    import concourse.tile as tile
    from concourse import bacc, mybir

    nt = t_len // TT
    nc = bacc.Bacc("TRN2", target_bir_lowering=False, debug=False, num_devices=1)
    f32 = mybir.dt.float32
    bf16 = mybir.dt.bfloat16
    add = mybir.AluOpType.add

    x1a_d = nc.dram_tensor("x1a", [CD1, t_len], bf16, kind="ExternalInput").ap()
    x1b_d = nc.dram_tensor("x1b", [CD2, t_len], bf16, kind="ExternalInput").ap()
    w1_d = nc.dram_tensor("wp1", [CD1, FREE], bf16, kind="ExternalInput").ap()
    w2_d = nc.dram_tensor("wp2", [CD2, FREE], bf16, kind="ExternalInput").ap()
    x2_d = nc.dram_tensor("x2h", [TT, nt * KI], bf16, kind="ExternalInput").ap()
    y_d = nc.dram_tensor("yout", [TT, nt * OH], f32, kind="ExternalOutput").ap()

    with tile.TileContext(nc) as tc:
        with (
            tc.tile_pool(name="const", bufs=1) as cpool,
            tc.tile_pool(name="ep", bufs=3) as epool,
            tc.tile_pool(name="tree", bufs=3) as tpool,
            tc.tile_pool(name="small", bufs=2) as spool,
            tc.tile_pool(name="psum", bufs=1, space="PSUM") as ppool,
        ):
            x1a = cpool.tile([CD1, t_len], bf16, tag="x1a")
            x1b = cpool.tile([CD2, t_len], bf16, tag="x1b")
            w1 = cpool.tile([CD1, FREE], bf16, tag="w1")
            w2 = cpool.tile([CD2, FREE], bf16, tag="w2")
            x2h = cpool.tile([TT, nt, KI], bf16, tag="x2h")
            y_sb = cpool.tile([TT, nt * OH], f32, tag="ysb")

            # Initial loads, split across the sync and gpsimd DMA queues with
            # the first tiles' dependencies (w chunks, x1 head, x2h head) first.
            h = t_len // 2
            hf = FREE // 2
            nc.sync.dma_start(x1a[:, 0:h], x1a_d[:, 0:h])
            nc.gpsimd.dma_start(x1b[:, 0:h], x1b_d[:, 0:h])
            nc.sync.dma_start(w1[:, 0:hf], w1_d[:, 0:hf])
            nc.gpsimd.dma_start(w2[:, 0:hf], w2_d[:, 0:hf])
            nc.sync.dma_start(w1[:, hf:], w1_d[:, hf:])
            nc.gpsimd.dma_start(w2[:, hf:], w2_d[:, hf:])
            nhalf = (nt // 2) * KI
            nc.sync.dma_start(
                x2h[:].rearrange("p a b -> p (a b)")[:, 0:nhalf], x2_d[:, 0:nhalf]
            )
            nc.gpsimd.dma_start(
                x2h[:].rearrange("p a b -> p (a b)")[:, nhalf:], x2_d[:, nhalf:]
            )
            nc.sync.dma_start(x1a[:, h:], x1a_d[:, h:])
            nc.gpsimd.dma_start(x1b[:, h:], x1b_d[:, h:])

            for tt in range(nt):
                t0 = tt * TT
                x1at = x1a[:, t0 : t0 + TT]
                x1bt = x1b[:, t0 : t0 + TT]

                # psum: 3 double-bank groups + 1 single bank (7 chunks of 512)
                pg = [
                    ppool.tile([TT, 1024], f32, tag="pA", name="pA"),
                    ppool.tile([TT, 1024], f32, tag="pB", name="pB"),
                    ppool.tile([TT, 1024], f32, tag="pC", name="pC"),
                    ppool.tile([TT, 512], f32, tag="pD", name="pD"),
                ]

                def chunk_ap(ci):
                    g, o = divmod(ci, 2)
                    return pg[g][:, o * 512 : (o + 1) * 512]

                # Interleave the two contraction halves per chunk so each
                # chunk (and its act) completes as early as possible.
                for ci in range(K):
                    cs = slice(ci * CHUNK, (ci + 1) * CHUNK)
                    nc.tensor.matmul(
                        chunk_ap(ci), x1at, w1[:, cs], start=True, stop=False
                    )
                    nc.tensor.matmul(
                        chunk_ap(ci), x1bt, w2[:, cs], start=False, stop=True
                    )

                # eex[t, k, {e, ex}, (o,i)]
                eex = epool.tile([TT, K, 2, SLAB], bf16, tag="eex")
                for g in range(4):
                    kw = 2 if g < 3 else 1  # k-slabs in this group
                    src = pg[g][:].rearrange("p (k q) -> p k q", k=kw)
                    nc.scalar.activation(
                        eex[:, 2 * g : 2 * g + kw, 0, :],
                        src,
                        mybir.ActivationFunctionType.Exp,
                    )

                # EX = e * x_unf broadcast over o (one wide 2x TT op)
                x24 = (
                    x2h[:, tt, :]
                    .rearrange("p (k i) -> p k i", k=K)
                    .unsqueeze(2)
                    .broadcast_to([TT, K, OH, C])
                )
                e4 = eex[:, :, 0, :].rearrange("p k (o i) -> p k o i", o=OH)
                ex4 = eex[:, :, 1, :].rearrange("p k (o i) -> p k o i", o=OH)
                nc.vector.tensor_mul(ex4, e4, x24)

                # k-sum trees for den (over e) and num (over EX), both halves
                # ride in each wide op via the (sn, q) flattening.
                ev = eex[:].rearrange("p k s q -> p k (s q)")
                t1 = tpool.tile([TT, 3, 2 * SLAB], bf16, tag="t1")
                nc.vector.tensor_add(t1[:], ev[:, 0:6:2], ev[:, 1:6:2])
                t2 = tpool.tile([TT, 2 * SLAB], bf16, tag="t2")
                nc.vector.tensor_add(t2[:], t1[:, 0], t1[:, 1])
                t3 = tpool.tile([TT, 2 * SLAB], bf16, tag="t3")
                nc.vector.tensor_add(t3[:], t1[:, 2], ev[:, 6])
                if tt % 2 == 0:
                    dn2 = spool.tile([TT, 2, SLAB], bf16, tag="dn2")
                    denf = spool.tile([TT, 2, SLAB], f32, tag="denf")
                # den goes straight to f32 (reciprocal needs it); num stays bf16
                nc.vector.tensor_add(denf[:, tt % 2], t2[:, 0:SLAB], t3[:, 0:SLAB])
                nc.vector.tensor_add(dn2[:, tt % 2], t2[:, SLAB:], t3[:, SLAB:])

                if tt % 2 == 1:
                    # softmax tail for the tile pair: r = 1/den, then
                    # y[t,o] = sum_i num * r (with one 2x i-halving first)
                    r2 = spool.tile([TT, 2, SLAB], f32, tag="r2")
                    nc.vector.reciprocal_approx_fast(out=r2[:], in_=denf[:])
                    y1 = spool.tile([TT, 2, SLAB], bf16, tag="y1")
                    nc.vector.tensor_mul(y1[:], dn2[:], r2[:])
                    y4 = y1[:].rearrange("p u (o h i) -> p u o h i", o=OH, h=2)
                    yh = spool.tile([TT, 2, OH, C // 2], bf16, tag="yh")
                    nc.vector.tensor_add(yh[:], y4[:, :, :, 0], y4[:, :, :, 1])
                    nc.vector.tensor_reduce(
                        y_sb[:, (tt - 1) * OH : (tt + 1) * OH],
                        yh[:],
                        axis=mybir.AxisListType.X,
                        op=mybir.AluOpType.add,
                    )

                if (tt + 1) % 8 == 0 or tt == nt - 1:
                    g0 = (tt // 8) * 8 * OH
                    nc.gpsimd.dma_start(
                        y_d[:, g0 : (tt + 1) * OH], y_sb[:, g0 : (tt + 1) * OH]
                    )

    nc.compile()
    return nc


def _prep_inputs(x, W, b):
    """Host-side scatter: per-core input dicts (pure layout/slicing)."""
    import ml_dtypes

    bf = ml_dtypes.bfloat16
    scale = np.float32(1.0 / np.sqrt(K))
    halves = []
    for h in range(2):
        Wh = W[h * OH * C * K : (h + 1) * OH * C * K]  # [OH*C*K, C, K]
        # rows (j,c) -> j*32+c ; cols (k,o,i) -> k*512 + o*32 + i
        Wp = (
            Wh.reshape(OH, C, K, C, K).transpose(4, 3, 2, 0, 1).reshape(K * C, FREE)
            * scale
        )
        bh = (
            b[h * OH * C * K : (h + 1) * OH * C * K]
            .reshape(OH, C, K)
            .transpose(2, 0, 1)
            .reshape(FREE)
            * scale
        )
        w1 = np.ascontiguousarray(Wp[:CD1])
        w2 = np.ascontiguousarray(
            np.concatenate([Wp[CD1:], bh[None, :]], axis=0)
        )
        halves.append((w1.astype(bf), w2.astype(bf)))

    t_len = x.shape[-1]
    nt = t_len // TT
    x1s = []
    for bi in range(B):
        xp = np.zeros((C, t_len + 2 * PAD), dtype=np.float32)
        xp[:, PAD : PAD + t_len] = x[bi]
        x1a = np.empty((CD1, t_len), dtype=np.float32)
        x1b = np.empty((CD2, t_len), dtype=np.float32)
        for j in range(K):
            tgt, r0 = (x1a, j * C) if j < 4 else (x1b, (j - 4) * C)
            tgt[r0 : r0 + C] = xp[:, j : j + t_len]
        x1b[CD2 - 1] = 1.0
        # x_unf in [t, (k,i)] order, tiled as [tp, tt, k*32+i]
        xu = np.empty((K, C, t_len), dtype=np.float32)
        for k in range(K):
            xu[k] = xp[:, k : k + t_len]
        x2h = (
            xu.transpose(2, 0, 1)  # [t, k, i]
            .reshape(nt, TT, KI)
            .transpose(1, 0, 2)  # [tp, tt, (k,i)]
            .reshape(TT, nt * KI)
        )
        x1s.append((x1a.astype(bf), x1b.astype(bf), np.ascontiguousarray(x2h).astype(bf)))

    in_maps = []
    for core in range(8):
        bi, h = divmod(core, 2)
        w1, w2 = halves[h]
        x1a, x1b, x2h = x1s[bi]
        in_maps.append({"x1a": x1a, "x1b": x1b, "wp1": w1, "wp2": w2, "x2h": x2h})
    return in_maps


def _assemble(results, t_len):
    """Gather per-core [TT, nt*OH] outputs into [B, O_FULL, t_len]."""
    nt = t_len // TT
    y = np.empty((B, O_FULL, t_len), dtype=np.float32)
    for core, res in enumerate(results):
        bi, h = divmod(core, 2)
        arr = res["yout"].reshape(TT, nt, OH)  # [tp, tt, o]
        y[bi, h * OH : (h + 1) * OH, :] = arr.transpose(2, 1, 0).reshape(OH, t_len)
    return y


def _run(x, W, b, trace=False, trace_cores=None):
    from concourse.bass_utils import run_bass_kernel_spmd
    from concourse.bass_interp import get_hw_module

    t_len = x.shape[-1]
    key = ("prog", t_len)
    if key not in _prog_cache:
        nc = _build(t_len)
        nc.m = get_hw_module(nc.m)
        _prog_cache[key] = nc
    nc = _prog_cache[key]

    in_maps = _prep_inputs(x, W, b)
    res = run_bass_kernel_spmd(
        nc,
        in_maps,
        core_ids=list(range(8)),
        trace=trace,
        trace_cores=trace_cores,
    )
    return _assemble(res.results, t_len), res


def kernel(x, W, b):
    y, _ = _run(np.asarray(x), np.asarray(W), np.asarray(b))
    return y
